# revision 14
# baseline (speedup 1.0000x reference)
"""Multi-head attention (B=4, T=2048, D=1024, H=16) on 8 TRN2 NeuronCores.

Sharding: core c -> (batch b = c//2, head-group g = c%2 of 8 heads).
Each core computes the qkv projection for its batch restricted to its 8
heads, full attention for those heads, and a partial output projection
(ctx_local @ Wout[rows of its heads]).  Host sums the two partials per batch.

Per-core kernel: a single flat software pipeline over 256 attention chunks
(4 head-pairs x 4 query-quarters x 16 k-chunks).  Per chunk: an S pair (two
concurrent 64-row-tile matmuls, one per head), exp on ACT, and the AV pair
deferred 3 chunks.  All projection work (qk for later pairs, v, output) is
broken into ~215ns steps and scheduled into specific chunks so the PE stream
stays ahead of ACT, which is the binding engine (~1us per chunk).  x stays
resident in SBUF; DMAs are issued in first-use order so attention starts
~11us in instead of ~43us.
"""

import numpy as np
import ml_dtypes
from contextlib import ExitStack

import concourse.bass as bass
import concourse.bacc as bacc
import concourse.tile as tile
from concourse import mybir
from concourse.bass_utils import run_bass_kernel_spmd

FP32 = mybir.dt.float32
BF16 = mybir.dt.bfloat16
EXP = mybir.ActivationFunctionType.Exp

D = 1024
T = 2048
HPC = 8          # heads per core
FC = 8           # feature chunks of 128 (projection contraction)
KC = 16          # k chunks of 128 per quarter
NG = 256         # total chunks: 4 pairs x 4 quarters x 16
AVD = 3          # AV defer (chunks)


def _norm(nc, rpool, ctx_sb, ctxp, hh, hc, qsl):
    """ctx_sb[hb:hb+64, hc, qsl] = ctxp[0:64] / ctxp[64] (sumexp row)."""
    hb = (hh % 2) * 64
    rtmp = rpool.tile([1, 512], FP32, tag="rtmp")
    nc.vector.tensor_copy(out=rtmp[:], in_=ctxp[64:65, :])
    rt = rpool.tile([1, 512], FP32, tag="rt")
    nc.vector.reciprocal_approx_fast(out=rt[:], in_=rtmp[:])
    rb = rpool.tile([64, 512], FP32, tag="rb")
    nc.gpsimd.partition_broadcast(rb[:], rt[0:1, :], channels=64)
    nc.vector.tensor_mul(ctx_sb[hb:hb + 64, hc, qsl], ctxp[0:64, :], rb[:])


def _body(ctx, nc, tc, xt_d, wq_d, wk_d, wv_d, wo_d, out_d):
    xt_r = xt_d.rearrange("(f p) t -> p f t", p=128)
    persist = ctx.enter_context(tc.tile_pool(name="persist", bufs=1))
    xbig = persist.tile([128, FC, T], BF16, tag="x")
    qT = persist.tile([128, 4, T], BF16, tag="qT")
    kT = persist.tile([128, 4, T], BF16, tag="kT")
    v_sb = persist.tile([128, KC, HPC, 65], BF16, tag="v")
    ctx_sb = persist.tile([128, 4, T], BF16, tag="ctx")
    wq_sb = persist.tile([128, FC, 512], BF16, tag="wq")
    wk_sb = persist.tile([128, FC, 512], BF16, tag="wk")
    wv_sb = persist.tile([128, FC, 512], BF16, tag="wv")
    wo_sb = persist.tile([128, 4, D], BF16, tag="wo")
    # circular exp-output buffer, indexed by global chunk mod NP2; reuse
    # distance (20 chunks) far exceeds the AV defer + quarter-trailing window.
    P2big = persist.tile([128, 12, 2, 512], BF16, tag="P2big")
    opstage = persist.tile([128, 16, D], BF16, tag="opstage")
    warm = persist.tile([1, 4], FP32, tag="warm")

    # Preload the ACT exp table-set during the initial DMA wait.
    nc.vector.memset(warm[:], 0.0)
    nc.scalar.activation(out=warm[:], in_=warm[:], func=EXP)

    nc.vector.memset(v_sb[:, :, :, 64:65], 1.0)

    # DMA in first-use order: wq+x0 (span-0 q proj), wk (span-0 k proj),
    # wv lo-half (v heads 0-3 JIT in the first quarter), x spans 1-3
    # (k-span proj at chunks 0-11), wv hi-half, wout (used from g=208).
    wq_r = wq_d.rearrange("(f p) c -> p f c", p=128)
    wk_r = wk_d.rearrange("(f p) c -> p f c", p=128)
    wv_r = wv_d.rearrange("(f p) c -> p f c", p=128)
    nc.sync.dma_start(out=wq_sb[:, 0:4, :], in_=wq_r[:, 0:4, :])
    nc.sync.dma_start(out=xbig[:, 0:4, 0:512], in_=xt_r[:, 0:4, 0:512])
    nc.sync.dma_start(out=wq_sb[:, 4:8, :], in_=wq_r[:, 4:8, :])
    nc.sync.dma_start(out=xbig[:, 4:8, 0:512], in_=xt_r[:, 4:8, 0:512])
    nc.sync.dma_start(out=wk_sb[:], in_=wk_r[:])
    nc.sync.dma_start(out=wv_sb[:, :, 0:256], in_=wv_r[:, :, 0:256])
    nc.sync.dma_start(out=xbig[:, :, 512:1024], in_=xt_r[:, :, 512:1024])
    nc.sync.dma_start(out=xbig[:, :, 1024:1536], in_=xt_r[:, :, 1024:1536])
    nc.sync.dma_start(out=wv_sb[:, :, 256:512], in_=wv_r[:, :, 256:512])
    nc.sync.dma_start(out=xbig[:, :, 1536:2048], in_=xt_r[:, :, 1536:2048])
    nc.sync.dma_start(out=wo_sb[:], in_=wo_d.rearrange("(c p) d -> p c d", p=128))

    spsum = ctx.enter_context(tc.tile_pool(name="spsum", bufs=2, space="PSUM"))
    cpsum = ctx.enter_context(tc.tile_pool(name="cpsum", bufs=2, space="PSUM"))
    # 8 PSUM banks total: spsum 4, ctx ring 2, unified projection ring 2.
    ps_u = ctx.enter_context(tc.tile_pool(name="ps_u", bufs=2, space="PSUM"))
    rpool = ctx.enter_context(tc.tile_pool(name="rpool", bufs=2))
    osb = ctx.enter_context(tc.tile_pool(name="osb", bufs=2))

    # ---- projection units ----
    # Every unit is atomic within one chunk (<=860ns of PE), accumulates in a
    # [128,256] fp32 slot of the shared 2-bank ring, and ends with one copy.

    def qk_burst(pair, ts, qk):
        """Immediate full-span projection of one (pair, span, q-or-k)."""
        tsl = slice(ts * 512, (ts + 1) * 512)
        w_sb, dst = ((wq_sb, qT), (wk_sb, kT))[qk]
        p = ps_u.tile([128, 512], FP32, tag="acc", name="qkburst")
        for fc in range(FC):
            nc.tensor.matmul(
                p[:],
                lhsT=w_sb[:, fc, pair * 128:(pair + 1) * 128],
                rhs=xbig[:, fc, tsl],
                start=(fc == 0), stop=(fc == FC - 1))
        nc.vector.tensor_copy(out=dst[:, pair, tsl], in_=p[:])

    def make_op_early(q):
        """cc 0-1 partial of (tcg, colq), staged to SBUF bf16; runnable as
        soon as head-pairs 0 and 1 have normed quarter q (g ~ 82+16q)."""
        def unit(s):
            unit_i, colq = divmod(s, 4)
            tcg = q * 4 + unit_i
            csl = slice(colq * 256, (colq + 1) * 256)
            po = ps_u.tile([128, 512], FP32, tag="acc", name="poe")
            for cc in (0, 1):
                nc.tensor.matmul(
                    po[:, 0:256],
                    lhsT=ctx_sb[:, cc, tcg * 128:(tcg + 1) * 128],
                    rhs=wo_sb[:, cc, csl],
                    start=(cc == 0), stop=(cc == 1))
            nc.vector.tensor_copy(out=opstage[:, tcg, csl], in_=po[:, 0:256])
        return unit

    def make_op_late(q):
        """cc 2-3 partial + staged add + DMA; needs head-pair 3's norm."""
        st = {"ot": None}

        def unit(s):
            unit_i, colq = divmod(s, 4)
            tcg = q * 4 + unit_i
            csl = slice(colq * 256, (colq + 1) * 256)
            if colq == 0:
                st["ot"] = osb.tile([128, D], FP32, tag="ot", name="ot")
            po = ps_u.tile([128, 512], FP32, tag="acc", name="pol")
            for cc in (2, 3):
                nc.tensor.matmul(
                    po[:, 0:256],
                    lhsT=ctx_sb[:, cc, tcg * 128:(tcg + 1) * 128],
                    rhs=wo_sb[:, cc, csl],
                    start=(cc == 2), stop=(cc == 3))
            nc.vector.tensor_add(
                out=st["ot"][:, csl], in0=opstage[:, tcg, csl],
                in1=po[:, 0:256])
            if colq == 3:
                nc.sync.dma_start(
                    out=out_d[tcg * 128:(tcg + 1) * 128, :],
                    in_=st["ot"][:])
        return unit

    qk_acc = {}

    def qk_half(pair, ts, qk, half, fchalf):
        """4-fc half of a 256-token projection; accumulator spans 2 chunks."""
        tsl = slice(ts * 512 + half * 256, ts * 512 + half * 256 + 256)
        w_sb, dst = ((wq_sb, qT), (wk_sb, kT))[qk]
        key = (pair, ts, qk, half)
        if fchalf == 0:
            qk_acc[key] = ps_u.tile([128, 512], FP32, tag="acc", name="qkp")
        p = qk_acc.pop(key) if fchalf == 1 else qk_acc[key]
        for fc in range(4 * fchalf, 4 * fchalf + 4):
            nc.tensor.matmul(
                p[:, 0:256],
                lhsT=w_sb[:, fc, pair * 128:(pair + 1) * 128],
                rhs=xbig[:, fc, tsl],
                start=(fc == 0), stop=(fc == FC - 1))
        if fchalf == 1:
            nc.vector.tensor_copy(out=dst[:, pair, tsl], in_=p[:, 0:256])

    v_acc = {}

    def v_unit(kc, lo):
        """v projection for k-chunk kc, heads 0-3 (lo), single chunk."""
        psv = ps_u.tile([128, 512], FP32, tag="acc", name="psv")
        xc = slice((kc // 4) * 512 + (kc % 4) * 128,
                   (kc // 4) * 512 + (kc % 4) * 128 + 128)
        for fc in range(FC):
            nc.tensor.matmul(
                psv[:, 0:256],
                lhsT=xbig[:, fc, xc],
                rhs=wv_sb[:, fc, 0:256],
                start=(fc == 0), stop=(fc == FC - 1))
        nc.vector.tensor_copy(
            out=v_sb[:, kc, 0:4, 0:64],
            in_=psv[:, 0:256].rearrange("p (h d) -> p h d", h=4))

    def v_half(kc, fchalf):
        """4-fc half of the heads-4-7 v projection for k-chunk kc."""
        if fchalf == 0:
            v_acc[kc] = ps_u.tile([128, 512], FP32, tag="acc", name="psv")
        psv = v_acc.pop(kc) if fchalf == 1 else v_acc[kc]
        xc = slice((kc // 4) * 512 + (kc % 4) * 128,
                   (kc // 4) * 512 + (kc % 4) * 128 + 128)
        for fc in range(4 * fchalf, 4 * fchalf + 4):
            nc.tensor.matmul(
                psv[:, 0:256],
                lhsT=xbig[:, fc, xc],
                rhs=wv_sb[:, fc, 256:512],
                start=(fc == 0), stop=(fc == FC - 1))
        if fchalf == 1:
            nc.vector.tensor_copy(
                out=v_sb[:, kc, 4:8, 0:64],
                in_=psv[:, 0:256].rearrange("p (h d) -> p h d", h=4))

    def make_op_unit(qq_prev):
        """2-cc half-units of the output projection of qq_prev's tokens.
        s in 0..31: (tcg, colq) unit = s//2, cc pair = s%2."""
        st = {"ot": None, "po": None}

        def unit(s):
            u, cchalf = divmod(s, 2)
            unit_i, colq = divmod(u, 4)
            tcg = qq_prev * 4 + unit_i
            csl = slice(colq * 256, (colq + 1) * 256)
            if cchalf == 0:
                if colq == 0:
                    st["ot"] = osb.tile([128, D], FP32, tag="ot", name="ot")
                st["po"] = ps_u.tile([128, 512], FP32, tag="acc", name="po")
            for cc in (0, 1) if cchalf == 0 else (2, 3):
                nc.tensor.matmul(
                    st["po"][:, 0:256],
                    lhsT=ctx_sb[:, cc, tcg * 128:(tcg + 1) * 128],
                    rhs=wo_sb[:, cc, csl],
                    start=(cc == 0), stop=(cc == 3))
            if cchalf == 1:
                nc.vector.tensor_copy(out=st["ot"][:, csl], in_=st["po"][:, 0:256])
                if colq == 3:
                    nc.sync.dma_start(
                        out=out_d[tcg * 128:(tcg + 1) * 128, :],
                        in_=st["ot"][:])
        return unit

    # ---- schedule    # ---- schedule: extra PE work per global chunk g ----

    sched = {g: [] for g in range(NG)}
    extra = [0.0] * NG          # scheduled extra PE ns per chunk

    def place(start, deadline, cost, fn, args, after=None):
        """Greedy: put the item on the least-loaded chunk in its window."""
        lo = max(start, after if after is not None else 0, 0)
        hi = min(deadline, NG - 1)
        g = min(range(lo, hi + 1), key=lambda x: (extra[x], x))
        sched[g].append((fn, args))
        extra[g] += cost
        return g

    def place_unit(start, deadline, cost, fn, base_args):
        """Two fc-halves in order; second within 2 chunks of the first."""
        g0 = place(start, deadline, cost, fn, base_args + (0,))
        place(g0, min(g0 + 2, deadline + 1), cost, fn, base_args + (1,),
              after=g0)

    # v heads 0-3: JIT at chunk kc (AV(kc) is emitted at chunk kc+AVD).
    for kc in range(KC):
        sched[kc].append((v_unit, (kc, True)))
        extra[kc] += 900
    # pair 0 remaining spans (k needed at 4*ts, q at 16*ts).
    for ts in range(1, 4):
        for half in range(2):
            place_unit(0, 4 * ts - 2, 440, qk_half, (0, ts, 1, half))
    for ts in range(1, 4):
        for half in range(2):
            place_unit(2, 16 * ts - 2, 440, qk_half, (0, ts, 0, half))
    # v heads 4-7 by g64.
    for kc in range(KC):
        place_unit(5, 61, 440, v_half, (kc,))
    # pairs 1-3: k spans by pair start, q span ts by its quarter.
    for pair in range(1, 4):
        for ts in range(4):
            for half in range(2):
                place_unit(5, 64 * pair + 4 * ts - 3, 440,
                           qk_half, (pair, ts, 1, half))
        for ts in range(4):
            for half in range(2):
                place_unit(5, 64 * pair + 16 * ts - 3, 440,
                           qk_half, (pair, ts, 0, half))
    # output projection cc0-1 partials: after head-pair 1's norm of the
    # token quarter (g ~ 82+16q); cc2-3 + finalize after head-pair 3's.
    for q in range(4):
        ope = make_op_early(q)
        for s in range(16):
            place(84 + 16 * q, 200, 330, ope, (s,))
    for q in range(3):
        opl = make_op_late(q)
        gp = None
        for s in range(16):
            # keep s in emission order: ot staging requires colq sequence
            gp = place(212 + 16 * q, min(211 + 16 * (q + 1), 255), 500,
                       opl, (s,), after=gp)

    # ---- the flat attention pipeline ----

    qk_burst(0, 0, 0)
    qk_burst(0, 0, 1)

    qinfo = {}

    def emit_av(j):
        info = qinfo[j // 16]
        kc = j % 16
        for i, ctxp in ((0, info["ctxA"]), (1, info["ctxB"])):
            nc.tensor.matmul(
                ctxp[:],
                lhsT=v_sb[:, kc, 2 * info["hc"] + i, :],
                rhs=P2big[:, j % 12, i, :],
                start=(kc == 0), stop=(kc == KC - 1))

    def emit_norm(q):
        info = qinfo[q]
        qsl = slice(info["qq"] * 512, (info["qq"] + 1) * 512)
        _norm(nc, rpool, ctx_sb, info["ctxA"], 2 * info["hc"], info["hc"], qsl)
        _norm(nc, rpool, ctx_sb, info["ctxB"], 2 * info["hc"] + 1,
              info["hc"], qsl)
        del qinfo[q]

    for g in range(NG):
        hc, qq, kc = g // 64, (g // 16) % 4, g % 16
        if kc == 0:
            qinfo[g // 16] = {
                "hc": hc, "qq": qq,
                "ctxA": cpsum.tile([65, 512], FP32, tag="ctx", name="ctxA"),
                "ctxB": cpsum.tile([65, 512], FP32, tag="ctx", name="ctxB"),
            }
        qsl = slice(qq * 512, (qq + 1) * 512)
        sps = spsum.tile([128, 2, 512], FP32, tag="S")
        for i in range(2):          # head A on rows 0-63, head B on 64-127
            b0 = i * 64
            nc.tensor.matmul(
                sps[:, i, :],
                lhsT=kT[b0:b0 + 64, hc, kc * 128:(kc + 1) * 128],
                rhs=qT[b0:b0 + 64, hc, qsl],
                start=True, stop=True)
        nc.scalar.activation(
            out=P2big[:, g % 12, :, :], in_=sps[:], func=EXP, scale=0.125)
        for fn, args in sched[g]:
            fn(*args)
        if g >= AVD:
            emit_av(g - AVD)
            if (g - AVD) % 16 == 15:
                emit_norm((g - AVD) // 16)
    for j in range(NG - AVD, NG):
        emit_av(j)
    emit_norm(15)
    # tail: output projection for the last quarter
    opl = make_op_late(3)
    for s in range(16):
        opl(s)


def build():
    nc = bacc.Bacc("TRN2", target_bir_lowering=False, debug=False, num_devices=8)
    xt_d = nc.dram_tensor("xt", [D, T], BF16, kind="ExternalInput").ap()
    wq_d = nc.dram_tensor("wq", [D, 512], BF16, kind="ExternalInput").ap()
    wk_d = nc.dram_tensor("wk", [D, 512], BF16, kind="ExternalInput").ap()
    wv_d = nc.dram_tensor("wv", [D, 512], BF16, kind="ExternalInput").ap()
    wo_d = nc.dram_tensor("wout", [512, D], BF16, kind="ExternalInput").ap()
    out_d = nc.dram_tensor("out", [T, D], FP32, kind="ExternalOutput").ap()
    with tile.TileContext(nc) as tc:
        with ExitStack() as ctx:
            _body(ctx, nc, tc, xt_d, wq_d, wk_d, wv_d, wo_d, out_d)
    nc.compile()
    return nc


_nc = None


def _get_nc():
    global _nc
    if _nc is None:
        _nc = build()
    return _nc


def make_in_maps(x, Wqkv, Wout):
    bf = ml_dtypes.bfloat16
    in_maps = []
    for c in range(8):
        b, g = divmod(c, 2)
        cs = slice(g * 512, (g + 1) * 512)
        in_maps.append({
            "xt": np.ascontiguousarray(x[b].T).astype(bf),
            "wq": np.ascontiguousarray(Wqkv[:, 0 * D:1 * D][:, cs]).astype(bf),
            "wk": np.ascontiguousarray(Wqkv[:, 1 * D:2 * D][:, cs]).astype(bf),
            "wv": np.ascontiguousarray(Wqkv[:, 2 * D:3 * D][:, cs]).astype(bf),
            "wout": np.ascontiguousarray(Wout[cs, :]).astype(bf),
        })
    return in_maps


def kernel(x, Wqkv, Wout, _trace=False):
    nc = _get_nc()
    x = np.asarray(x, dtype=np.float32)
    Wqkv = np.asarray(Wqkv, dtype=np.float32)
    Wout = np.asarray(Wout, dtype=np.float32)
    in_maps = make_in_maps(x, Wqkv, Wout)
    kwargs = {}
    if _trace:
        kwargs["trace"] = True
    res = run_bass_kernel_spmd(nc, in_maps, core_ids=list(range(8)), **kwargs)
    outs = [res.results[c]["out"] for c in range(8)]
    out = np.stack([outs[2 * b] + outs[2 * b + 1] for b in range(4)])
    if _trace:
        kernel.last_result = res
    return out


# revision 15
# speedup vs baseline: 1.0067x; 1.0067x over previous
"""Multi-head attention (B=4, T=2048, D=1024, H=16) on 8 TRN2 NeuronCores.

Sharding: core c -> (batch b = c//2, head-group g = c%2 of 8 heads).
Each core computes the qkv projection for its batch restricted to its 8
heads, full attention for those heads, and a partial output projection
(ctx_local @ Wout[rows of its heads]).  Host sums the two partials per batch.

Per-core kernel: the PE stream is the bottleneck (~320us incl. the ~105ns
drain paid when entering/leaving the split-tile S-pairs), so the schedule
minimizes matmul class switches: superchunks of 2 attention chunks emit
[S,S][AV x4][one free-512 projection burst].  The two S matmuls of a chunk
run concurrently on disjoint 64-row PE tiles (one per head); exp runs on the
ACT engine (286us total, slack vs PE); AV is deferred 4 chunks through a
circular bf16 P buffer.  The output projection is split by contraction: the
head-pair-0/1 partial runs early (staged bf16), pair-2/3 + finalize after
the last norms, shrinking the tail.  x stays resident in SBUF; DMAs are
issued in first-use order so the first matmul lands ~13us in.
"""

import numpy as np
import ml_dtypes
from contextlib import ExitStack

import concourse.bass as bass
import concourse.bacc as bacc
import concourse.tile as tile
from concourse import mybir
from concourse.bass_utils import run_bass_kernel_spmd

FP32 = mybir.dt.float32
BF16 = mybir.dt.bfloat16
EXP = mybir.ActivationFunctionType.Exp

D = 1024
T = 2048
HPC = 8          # heads per core
FC = 8           # feature chunks of 128 (projection contraction)
KC = 16          # k chunks of 128 per quarter
NG = 256         # total chunks: 4 pairs x 4 quarters x 16
AVD = 4          # AV defer (chunks)
NP2 = 12         # circular exp-output slots


def _norm(nc, rpool, ctx_sb, ctxp, hh, hc, qsl):
    """ctx_sb[hb:hb+64, hc, qsl] = ctxp[0:64] / ctxp[64] (sumexp row)."""
    hb = (hh % 2) * 64
    rtmp = rpool.tile([1, 512], FP32, tag="rtmp")
    nc.vector.tensor_copy(out=rtmp[:], in_=ctxp[64:65, :])
    rt = rpool.tile([1, 512], FP32, tag="rt")
    nc.vector.reciprocal_approx_fast(out=rt[:], in_=rtmp[:])
    rb = rpool.tile([64, 512], FP32, tag="rb")
    nc.gpsimd.partition_broadcast(rb[:], rt[0:1, :], channels=64)
    nc.vector.tensor_mul(ctx_sb[hb:hb + 64, hc, qsl], ctxp[0:64, :], rb[:])


def _body(ctx, nc, tc, xt_d, wq_d, wk_d, wv_d, wo_d, out_d):
    xt_r = xt_d.rearrange("(f p) t -> p f t", p=128)
    persist = ctx.enter_context(tc.tile_pool(name="persist", bufs=1))
    xbig = persist.tile([128, FC, T], BF16, tag="x")
    qT = persist.tile([128, 4, T], BF16, tag="qT")
    kT = persist.tile([128, 4, T], BF16, tag="kT")
    v_sb = persist.tile([128, KC, HPC, 65], BF16, tag="v")
    ctx_sb = persist.tile([128, 4, T], BF16, tag="ctx")
    wq_sb = persist.tile([128, FC, 512], BF16, tag="wq")
    wk_sb = persist.tile([128, FC, 512], BF16, tag="wk")
    wv_sb = persist.tile([128, FC, 512], BF16, tag="wv")
    wo_sb = persist.tile([128, 4, D], BF16, tag="wo")
    P2big = persist.tile([128, NP2, 2, 512], BF16, tag="P2big")
    opstage = persist.tile([128, 16, D], BF16, tag="opstage")
    warm = persist.tile([1, 4], FP32, tag="warm")

    # Preload the ACT exp table-set during the initial DMA wait.
    nc.vector.memset(warm[:], 0.0)
    nc.scalar.activation(out=warm[:], in_=warm[:], func=EXP)

    nc.vector.memset(v_sb[:, :, :, 64:65], 1.0)

    # DMA in first-use order.
    wq_r = wq_d.rearrange("(f p) c -> p f c", p=128)
    wk_r = wk_d.rearrange("(f p) c -> p f c", p=128)
    wv_r = wv_d.rearrange("(f p) c -> p f c", p=128)
    nc.sync.dma_start(out=wq_sb[:, 0:4, :], in_=wq_r[:, 0:4, :])
    nc.sync.dma_start(out=xbig[:, 0:4, 0:512], in_=xt_r[:, 0:4, 0:512])
    nc.sync.dma_start(out=wq_sb[:, 4:8, :], in_=wq_r[:, 4:8, :])
    nc.sync.dma_start(out=xbig[:, 4:8, 0:512], in_=xt_r[:, 4:8, 0:512])
    nc.sync.dma_start(out=wk_sb[:], in_=wk_r[:])
    nc.sync.dma_start(out=wv_sb[:], in_=wv_r[:])
    nc.sync.dma_start(out=xbig[:, :, 512:1024], in_=xt_r[:, :, 512:1024])
    nc.sync.dma_start(out=xbig[:, :, 1024:1536], in_=xt_r[:, :, 1024:1536])
    nc.sync.dma_start(out=xbig[:, :, 1536:2048], in_=xt_r[:, :, 1536:2048])
    nc.sync.dma_start(out=wo_sb[:], in_=wo_d.rearrange("(c p) d -> p c d", p=128))

    spsum = ctx.enter_context(tc.tile_pool(name="spsum", bufs=2, space="PSUM"))
    cpsum = ctx.enter_context(tc.tile_pool(name="cpsum", bufs=2, space="PSUM"))
    # 8 PSUM banks: spsum 4, ctx ring 2, unified projection ring 2.
    ps_u = ctx.enter_context(tc.tile_pool(name="ps_u", bufs=2, space="PSUM"))
    rpool = ctx.enter_context(tc.tile_pool(name="rpool", bufs=2))
    osb = ctx.enter_context(tc.tile_pool(name="osb", bufs=2))

    # ---- projection bursts (each one free-512 burst + one copy) ----

    def qk_burst(pair, ts, qk):
        """Full-span projection of one (pair, span, q-or-k)."""
        tsl = slice(ts * 512, (ts + 1) * 512)
        w_sb, dst = ((wq_sb, qT), (wk_sb, kT))[qk]
        p = ps_u.tile([128, 512], FP32, tag="acc", name="qkp")
        for fc in range(FC):
            nc.tensor.matmul(
                p[:],
                lhsT=w_sb[:, fc, pair * 128:(pair + 1) * 128],
                rhs=xbig[:, fc, tsl],
                start=(fc == 0), stop=(fc == FC - 1))
        nc.vector.tensor_copy(out=dst[:, pair, tsl], in_=p[:])

    def v_burst(kc):
        """v projection for k-chunk kc, all 8 heads."""
        psv = ps_u.tile([128, 512], FP32, tag="acc", name="psv")
        xc = slice((kc // 4) * 512 + (kc % 4) * 128,
                   (kc // 4) * 512 + (kc % 4) * 128 + 128)
        for fc in range(FC):
            nc.tensor.matmul(
                psv[:],
                lhsT=xbig[:, fc, xc],
                rhs=wv_sb[:, fc, :],
                start=(fc == 0), stop=(fc == FC - 1))
        nc.vector.tensor_copy(
            out=v_sb[:, kc, :, 0:64],
            in_=psv[:].rearrange("p (h d) -> p h d", h=HPC))

    def make_op_early(q):
        """cc 0-1 partial of (tcg, col-half), staged to SBUF bf16; runnable
        once head-pairs 0 and 1 have normed quarter q."""
        def unit(s):
            unit_i, ch = divmod(s, 2)
            tcg = q * 4 + unit_i
            csl = slice(ch * 512, (ch + 1) * 512)
            po = ps_u.tile([128, 512], FP32, tag="acc", name="poe")
            for cc in (0, 1):
                nc.tensor.matmul(
                    po[:],
                    lhsT=ctx_sb[:, cc, tcg * 128:(tcg + 1) * 128],
                    rhs=wo_sb[:, cc, csl],
                    start=(cc == 0), stop=(cc == 1))
            nc.vector.tensor_copy(out=opstage[:, tcg, csl], in_=po[:])
        return unit

    def make_op_late(q):
        """cc 2-3 partial + staged add + DMA; needs head-pair 3's norm."""
        st = {"ot": None}

        def unit(s):
            unit_i, ch = divmod(s, 2)
            tcg = q * 4 + unit_i
            csl = slice(ch * 512, (ch + 1) * 512)
            if ch == 0:
                st["ot"] = osb.tile([128, D], FP32, tag="ot", name="ot")
            po = ps_u.tile([128, 512], FP32, tag="acc", name="pol")
            for cc in (2, 3):
                nc.tensor.matmul(
                    po[:],
                    lhsT=ctx_sb[:, cc, tcg * 128:(tcg + 1) * 128],
                    rhs=wo_sb[:, cc, csl],
                    start=(cc == 2), stop=(cc == 3))
            nc.vector.tensor_add(
                out=st["ot"][:, csl], in0=opstage[:, tcg, csl], in1=po[:])
            if ch == 1:
                nc.sync.dma_start(
                    out=out_d[tcg * 128:(tcg + 1) * 128, :],
                    in_=st["ot"][:])
        return unit

    # ---- schedule: one burst list per superchunk of 2 chunks ----

    NSC = NG // 2
    sched = {s: [] for s in range(NSC)}
    extra = [0.0] * NSC

    def place(start, deadline, cost, fn, args, after=None):
        lo = max(start, after if after is not None else 0, 0)
        hi = min(deadline, NSC - 1)
        s = min(range(lo, hi + 1), key=lambda x: (extra[x], x))
        sched[s].append((fn, args))
        extra[s] += cost
        return s

    # v (all heads): JIT, chunks 2s and 2s+1 in quarter 0 (AV defer 4).
    for kc in range(KC):
        sched[kc // 2].append((v_burst, (kc,)))
        extra[kc // 2] += 1930
    # pair 0 remaining spans: k-span ts by chunk 4ts, q-span ts by 16ts.
    for ts in range(1, 4):
        place(0, 2 * ts - 1, 1930, qk_burst, (0, ts, 1))
    for ts in range(1, 4):
        place(1, 8 * ts - 2, 1930, qk_burst, (0, ts, 0))
    # pairs 1-3: k-span ts by chunk 64p+4ts, q-span ts by 64p+16ts.
    for pair in range(1, 4):
        for ts in range(4):
            place(3, 32 * pair + 2 * ts - 2, 1930, qk_burst, (pair, ts, 1))
        for ts in range(4):
            place(3, 32 * pair + 8 * ts - 2, 1930, qk_burst, (pair, ts, 0))
    # output projection cc0-1 after head-pair 1's norm (chunk ~82+16q);
    # cc2-3 + finalize after head-pair 3's norm (chunk ~210+16q).
    for q in range(4):
        ope = make_op_early(q)
        for s in range(8):
            place(43 + 8 * q, 101, 650, ope, (s,))
    for q in range(3):
        opl = make_op_late(q)
        sp = None
        for s in range(8):
            sp = place(107 + 8 * q, min(106 + 8 * (q + 1), NSC - 1), 650,
                       opl, (s,), after=sp)

    # ---- the flat attention pipeline ----

    qk_burst(0, 0, 0)
    qk_burst(0, 0, 1)

    qinfo = {}

    def emit_av(j):
        info = qinfo[j // 16]
        kc = j % 16
        for i, ctxp in ((0, info["ctxA"]), (1, info["ctxB"])):
            nc.tensor.matmul(
                ctxp[:],
                lhsT=v_sb[:, kc, 2 * info["hc"] + i, :],
                rhs=P2big[:, j % NP2, i, :],
                start=(kc == 0), stop=(kc == KC - 1))

    def emit_norm(q):
        info = qinfo[q]
        qsl = slice(info["qq"] * 512, (info["qq"] + 1) * 512)
        _norm(nc, rpool, ctx_sb, info["ctxA"], 2 * info["hc"], info["hc"], qsl)
        _norm(nc, rpool, ctx_sb, info["ctxB"], 2 * info["hc"] + 1,
              info["hc"], qsl)
        del qinfo[q]

    def emit_s_exp(g):
        hc, qq, kc = g // 64, (g // 16) % 4, g % 16
        if kc == 0:
            qinfo[g // 16] = {
                "hc": hc, "qq": qq,
                "ctxA": cpsum.tile([65, 512], FP32, tag="ctx", name="ctxA"),
                "ctxB": cpsum.tile([65, 512], FP32, tag="ctx", name="ctxB"),
            }
        qsl = slice(qq * 512, (qq + 1) * 512)
        sps = spsum.tile([128, 2, 512], FP32, tag="S")
        for i in range(2):          # head A on rows 0-63, head B on 64-127
            b0 = i * 64
            nc.tensor.matmul(
                sps[:, i, :],
                lhsT=kT[b0:b0 + 64, hc, kc * 128:(kc + 1) * 128],
                rhs=qT[b0:b0 + 64, hc, qsl],
                start=True, stop=True)
        nc.scalar.activation(
            out=P2big[:, g % NP2, :, :], in_=sps[:], func=EXP, scale=0.125)

    for sc in range(NSC):
        emit_s_exp(2 * sc)
        emit_s_exp(2 * sc + 1)
        for j in (2 * sc - AVD, 2 * sc + 1 - AVD):
            if j >= 0:
                emit_av(j)
                if j % 16 == 15:
                    emit_norm(j // 16)
        for fn, args in sched[sc]:
            fn(*args)
    for j in range(NG - AVD, NG):
        emit_av(j)
    emit_norm(15)
    # tail: output projection for the last quarter
    opl = make_op_late(3)
    for s in range(8):
        opl(s)


def build():
    nc = bacc.Bacc("TRN2", target_bir_lowering=False, debug=False, num_devices=8)
    xt_d = nc.dram_tensor("xt", [D, T], BF16, kind="ExternalInput").ap()
    wq_d = nc.dram_tensor("wq", [D, 512], BF16, kind="ExternalInput").ap()
    wk_d = nc.dram_tensor("wk", [D, 512], BF16, kind="ExternalInput").ap()
    wv_d = nc.dram_tensor("wv", [D, 512], BF16, kind="ExternalInput").ap()
    wo_d = nc.dram_tensor("wout", [512, D], BF16, kind="ExternalInput").ap()
    out_d = nc.dram_tensor("out", [T, D], FP32, kind="ExternalOutput").ap()
    with tile.TileContext(nc) as tc:
        with ExitStack() as ctx:
            _body(ctx, nc, tc, xt_d, wq_d, wk_d, wv_d, wo_d, out_d)
    nc.compile()
    return nc


_nc = None


def _get_nc():
    global _nc
    if _nc is None:
        _nc = build()
    return _nc


def make_in_maps(x, Wqkv, Wout):
    bf = ml_dtypes.bfloat16
    in_maps = []
    for c in range(8):
        b, g = divmod(c, 2)
        cs = slice(g * 512, (g + 1) * 512)
        in_maps.append({
            "xt": np.ascontiguousarray(x[b].T).astype(bf),
            "wq": np.ascontiguousarray(Wqkv[:, 0 * D:1 * D][:, cs]).astype(bf),
            "wk": np.ascontiguousarray(Wqkv[:, 1 * D:2 * D][:, cs]).astype(bf),
            "wv": np.ascontiguousarray(Wqkv[:, 2 * D:3 * D][:, cs]).astype(bf),
            "wout": np.ascontiguousarray(Wout[cs, :]).astype(bf),
        })
    return in_maps


def kernel(x, Wqkv, Wout, _trace=False):
    nc = _get_nc()
    x = np.asarray(x, dtype=np.float32)
    Wqkv = np.asarray(Wqkv, dtype=np.float32)
    Wout = np.asarray(Wout, dtype=np.float32)
    in_maps = make_in_maps(x, Wqkv, Wout)
    kwargs = {}
    if _trace:
        kwargs["trace"] = True
    res = run_bass_kernel_spmd(nc, in_maps, core_ids=list(range(8)), **kwargs)
    outs = [res.results[c]["out"] for c in range(8)]
    out = np.stack([outs[2 * b] + outs[2 * b + 1] for b in range(4)])
    if _trace:
        kernel.last_result = res
    return out


# revision 16
# speedup vs baseline: 1.0412x; 1.0342x over previous
"""Multi-head attention (B=4, T=2048, D=1024, H=16) on 8 TRN2 NeuronCores.

Sharding: core c -> (batch b = c//2, head-group g = c%2 of 8 heads).
Each core computes the qkv projection for its batch restricted to its 8
heads, full attention for those heads, and a partial output projection
(ctx_local @ Wout[rows of its heads]).  Host sums the two partials per batch.

Per-core kernel: the PE stream is the bottleneck (~320us incl. the ~105ns
drain paid when entering/leaving the split-tile S-pairs), so the schedule
minimizes matmul class switches: superchunks of 2 attention chunks emit
[S,S][AV x4][one free-512 projection burst].  The two S matmuls of a chunk
run concurrently on disjoint 64-row PE tiles (one per head); exp runs on the
ACT engine (286us total, slack vs PE); AV is deferred 4 chunks through a
circular bf16 P buffer.  The output projection is split by contraction: the
head-pair-0/1 partial runs early (staged bf16), pair-2/3 + finalize after
the last norms, shrinking the tail.  x stays resident in SBUF; DMAs are
issued in first-use order so the first matmul lands ~13us in.
"""

import numpy as np
import ml_dtypes
from contextlib import ExitStack

import concourse.bass as bass
import concourse.bacc as bacc
import concourse.tile as tile
from concourse import mybir
from concourse.bass_utils import run_bass_kernel_spmd

FP32 = mybir.dt.float32
BF16 = mybir.dt.bfloat16
EXP = mybir.ActivationFunctionType.Exp

D = 1024
T = 2048
HPC = 8          # heads per core
FC = 8           # feature chunks of 128 (projection contraction)
KC = 16          # k chunks of 128 per quarter
NG = 256         # total chunks: 4 pairs x 4 quarters x 16
AVD = 4          # AV defer (chunks)
NP2 = 12         # circular exp-output slots


def _norm(nc, rpool, ctx_sb, ctxp, hh, hc, qsl):
    """ctx_sb[hb:hb+64, hc, qsl] = ctxp[0:64] / ctxp[64] (sumexp row)."""
    hb = (hh % 2) * 64
    rtmp = rpool.tile([1, 512], FP32, tag="rtmp")
    nc.vector.tensor_copy(out=rtmp[:], in_=ctxp[64:65, :])
    rt = rpool.tile([1, 512], FP32, tag="rt")
    nc.vector.reciprocal_approx_fast(out=rt[:], in_=rtmp[:])
    rb = rpool.tile([64, 512], FP32, tag="rb")
    nc.gpsimd.partition_broadcast(rb[:], rt[0:1, :], channels=64)
    nc.vector.tensor_mul(ctx_sb[hb:hb + 64, hc, qsl], ctxp[0:64, :], rb[:])


def _body(ctx, nc, tc, xt_d, wq_d, wk_d, wv_d, wo_d, out_d):
    xt_r = xt_d.rearrange("(f p) t -> p f t", p=128)
    persist = ctx.enter_context(tc.tile_pool(name="persist", bufs=1))
    xbig = persist.tile([128, FC, T], BF16, tag="x")
    qT = persist.tile([128, 4, T], BF16, tag="qT")
    kT = persist.tile([128, 4, T], BF16, tag="kT")
    v_sb = persist.tile([128, KC, HPC, 65], BF16, tag="v")
    ctx_sb = persist.tile([128, 4, T], BF16, tag="ctx")
    wq_sb = persist.tile([128, FC, 512], BF16, tag="wq")
    wk_sb = persist.tile([128, FC, 512], BF16, tag="wk")
    wv_sb = persist.tile([128, FC, 512], BF16, tag="wv")
    wo_sb = persist.tile([128, 4, D], BF16, tag="wo")
    P2big = persist.tile([128, NP2, 2, 512], BF16, tag="P2big")
    opstage = persist.tile([128, 16, D], BF16, tag="opstage")
    warm = persist.tile([1, 4], FP32, tag="warm")

    # Preload the ACT exp table-set during the initial DMA wait.
    nc.vector.memset(warm[:], 0.0)
    nc.scalar.activation(out=warm[:], in_=warm[:], func=EXP)

    nc.vector.memset(v_sb[:, :, :, 64:65], 1.0)

    # DMA in first-use order.
    wq_r = wq_d.rearrange("(f p) c -> p f c", p=128)
    wk_r = wk_d.rearrange("(f p) c -> p f c", p=128)
    wv_r = wv_d.rearrange("(f p) c -> p f c", p=128)
    nc.sync.dma_start(out=wq_sb[:, 0:4, :], in_=wq_r[:, 0:4, :])
    nc.sync.dma_start(out=xbig[:, 0:4, 0:512], in_=xt_r[:, 0:4, 0:512])
    nc.sync.dma_start(out=wq_sb[:, 4:8, :], in_=wq_r[:, 4:8, :])
    nc.sync.dma_start(out=xbig[:, 4:8, 0:512], in_=xt_r[:, 4:8, 0:512])
    nc.sync.dma_start(out=wk_sb[:], in_=wk_r[:])
    nc.sync.dma_start(out=wv_sb[:], in_=wv_r[:])
    nc.sync.dma_start(out=xbig[:, :, 512:1024], in_=xt_r[:, :, 512:1024])
    nc.sync.dma_start(out=xbig[:, :, 1024:1536], in_=xt_r[:, :, 1024:1536])
    nc.sync.dma_start(out=xbig[:, :, 1536:2048], in_=xt_r[:, :, 1536:2048])
    nc.sync.dma_start(out=wo_sb[:], in_=wo_d.rearrange("(c p) d -> p c d", p=128))

    spsum = ctx.enter_context(tc.tile_pool(name="spsum", bufs=2, space="PSUM"))
    cpsum = ctx.enter_context(tc.tile_pool(name="cpsum", bufs=2, space="PSUM"))
    # 8 PSUM banks: spsum 4, ctx ring 2, unified projection ring 2.
    ps_u = ctx.enter_context(tc.tile_pool(name="ps_u", bufs=2, space="PSUM"))
    rpool = ctx.enter_context(tc.tile_pool(name="rpool", bufs=2))
    osb = ctx.enter_context(tc.tile_pool(name="osb", bufs=2))

    # ---- projection bursts (each one free-512 burst + one copy) ----

    def qk_burst(pair, ts, qk):
        """Full-span projection of one (pair, span, q-or-k)."""
        tsl = slice(ts * 512, (ts + 1) * 512)
        w_sb, dst = ((wq_sb, qT), (wk_sb, kT))[qk]
        p = ps_u.tile([128, 512], FP32, tag="acc", name="qkp")
        for fc in range(FC):
            nc.tensor.matmul(
                p[:],
                lhsT=w_sb[:, fc, pair * 128:(pair + 1) * 128],
                rhs=xbig[:, fc, tsl],
                start=(fc == 0), stop=(fc == FC - 1))
        nc.vector.tensor_copy(out=dst[:, pair, tsl], in_=p[:])

    def v_burst(kc, lo):
        """v projection for k-chunk kc, heads 0-3 (lo) or 4-7."""
        csl = slice(0, 256) if lo else slice(256, 512)
        psv = ps_u.tile([128, 512], FP32, tag="acc", name="psv")
        xc = slice((kc // 4) * 512 + (kc % 4) * 128,
                   (kc // 4) * 512 + (kc % 4) * 128 + 128)
        for fc in range(FC):
            nc.tensor.matmul(
                psv[:, 0:256],
                lhsT=xbig[:, fc, xc],
                rhs=wv_sb[:, fc, csl],
                start=(fc == 0), stop=(fc == FC - 1))
        h0 = 0 if lo else 4
        nc.vector.tensor_copy(
            out=v_sb[:, kc, h0:h0 + 4, 0:64],
            in_=psv[:, 0:256].rearrange("p (h d) -> p h d", h=4))

    def make_op_early(q):
        """cc 0-1 partial of (tcg, col-half), staged to SBUF bf16; runnable
        once head-pairs 0 and 1 have normed quarter q."""
        def unit(s):
            unit_i, ch = divmod(s, 2)
            tcg = q * 4 + unit_i
            csl = slice(ch * 512, (ch + 1) * 512)
            po = ps_u.tile([128, 512], FP32, tag="acc", name="poe")
            for cc in (0, 1):
                nc.tensor.matmul(
                    po[:],
                    lhsT=ctx_sb[:, cc, tcg * 128:(tcg + 1) * 128],
                    rhs=wo_sb[:, cc, csl],
                    start=(cc == 0), stop=(cc == 1))
            nc.vector.tensor_copy(out=opstage[:, tcg, csl], in_=po[:])
        return unit

    def make_op_mid(q):
        """cc 2 partial accumulated into the bf16 stage; needs pair 2."""
        def unit(s):
            unit_i, ch = divmod(s, 2)
            tcg = q * 4 + unit_i
            csl = slice(ch * 512, (ch + 1) * 512)
            po = ps_u.tile([128, 512], FP32, tag="acc", name="pom")
            nc.tensor.matmul(
                po[:],
                lhsT=ctx_sb[:, 2, tcg * 128:(tcg + 1) * 128],
                rhs=wo_sb[:, 2, csl],
                start=True, stop=True)
            nc.vector.tensor_add(
                out=opstage[:, tcg, csl], in0=opstage[:, tcg, csl], in1=po[:])
        return unit

    def make_op_late(q):
        """cc 3 partial + staged add + DMA; needs head-pair 3's norm."""
        st = {"ot": None}

        def unit(s):
            unit_i, ch = divmod(s, 2)
            tcg = q * 4 + unit_i
            csl = slice(ch * 512, (ch + 1) * 512)
            if ch == 0:
                st["ot"] = osb.tile([128, D], FP32, tag="ot", name="ot")
            po = ps_u.tile([128, 512], FP32, tag="acc", name="pol")
            nc.tensor.matmul(
                po[:],
                lhsT=ctx_sb[:, 3, tcg * 128:(tcg + 1) * 128],
                rhs=wo_sb[:, 3, csl],
                start=True, stop=True)
            nc.vector.tensor_add(
                out=st["ot"][:, csl], in0=opstage[:, tcg, csl], in1=po[:])
            if ch == 1:
                nc.sync.dma_start(
                    out=out_d[tcg * 128:(tcg + 1) * 128, :],
                    in_=st["ot"][:])
        return unit

    # ---- schedule: one burst list per superchunk of 2 chunks ----

    NSC = NG // 2
    sched = {s: [] for s in range(NSC)}
    extra = [0.0] * NSC

    def place(start, deadline, cost, fn, args, after=None):
        lo = max(start, after if after is not None else 0, 0)
        hi = min(deadline, NSC - 1)
        s = min(range(lo, hi + 1), key=lambda x: (extra[x], -x))
        sched[s].append((fn, args))
        extra[s] += cost
        return s

    # v heads 0-3: JIT in quarter 0 (AV defer 4); heads 4-7 by chunk 128.
    for kc in range(KC):
        sched[kc // 2].append((v_burst, (kc, True)))
        extra[kc // 2] += 1000
    for kc in range(KC):
        place(32, 62, 1000, v_burst, (kc, False))
    # pair 0 remaining spans: k-span ts by chunk 4ts, q-span ts by 16ts.
    for ts in range(1, 4):
        place(0, 2 * ts - 1, 1930, qk_burst, (0, ts, 1))
    for ts in range(1, 4):
        place(1, 8 * ts - 2, 1930, qk_burst, (0, ts, 0))
    # pairs 1-3: k-span ts by chunk 64p+4ts, q-span ts by 64p+16ts.
    for pair in range(1, 4):
        for ts in range(4):
            place(3, 32 * pair + 2 * ts - 2, 1930, qk_burst, (pair, ts, 1))
        for ts in range(4):
            place(3, 32 * pair + 8 * ts - 2, 1930, qk_burst, (pair, ts, 0))
    # output projection cc0-1 after head-pair 1's norm (chunk ~82+16q);
    # cc2-3 + finalize after head-pair 3's norm (chunk ~210+16q).
    for q in range(4):
        ope = make_op_early(q)
        opm = make_op_mid(q)
        for s in range(8):
            ge = place(43 + 8 * q, 101, 650, ope, (s,))
            place(max(75 + 8 * q, ge), 104, 450, opm, (s,), after=ge)
    for q in range(3):
        opl = make_op_late(q)
        sp = None
        for s in range(8):
            sp = place(107 + 8 * q, min(106 + 8 * (q + 1), NSC - 1), 450,
                       opl, (s,), after=sp)

    # ---- the flat attention pipeline ----

    def qk_burst_fchalf(qk, fchalf):
        tsl = slice(0, 512)
        w_sb, dst = ((wq_sb, qT), (wk_sb, kT))[qk]
        p = startacc[qk]
        for fc in range(4 * fchalf, 4 * fchalf + 4):
            nc.tensor.matmul(
                p[:],
                lhsT=w_sb[:, fc, 0:128],
                rhs=xbig[:, fc, tsl],
                start=(fc == 0), stop=(fc == FC - 1))
        if fchalf == 1:
            nc.vector.tensor_copy(out=dst[:, 0, tsl], in_=p[:])

    startacc = [ps_u.tile([128, 512], FP32, tag="acc", name="sa0"),
                ps_u.tile([128, 512], FP32, tag="acc", name="sa1")]
    qk_burst_fchalf(0, 0)
    qk_burst_fchalf(0, 1)
    qk_burst_fchalf(1, 0)
    qk_burst_fchalf(1, 1)

    qinfo = {}

    def emit_av(j):
        info = qinfo[j // 16]
        kc = j % 16
        for i, ctxp in ((0, info["ctxA"]), (1, info["ctxB"])):
            nc.tensor.matmul(
                ctxp[:],
                lhsT=v_sb[:, kc, 2 * info["hc"] + i, :],
                rhs=P2big[:, j % NP2, i, :],
                start=(kc == 0), stop=(kc == KC - 1))

    def emit_norm(q):
        info = qinfo[q]
        qsl = slice(info["qq"] * 512, (info["qq"] + 1) * 512)
        _norm(nc, rpool, ctx_sb, info["ctxA"], 2 * info["hc"], info["hc"], qsl)
        _norm(nc, rpool, ctx_sb, info["ctxB"], 2 * info["hc"] + 1,
              info["hc"], qsl)
        del qinfo[q]

    def emit_s_exp(g):
        hc, qq, kc = g // 64, (g // 16) % 4, g % 16
        if kc == 0:
            qinfo[g // 16] = {
                "hc": hc, "qq": qq,
                "ctxA": cpsum.tile([65, 512], FP32, tag="ctx", name="ctxA"),
                "ctxB": cpsum.tile([65, 512], FP32, tag="ctx", name="ctxB"),
            }
        qsl = slice(qq * 512, (qq + 1) * 512)
        sps = spsum.tile([128, 2, 512], FP32, tag="S")
        for i in range(2):          # head A on rows 0-63, head B on 64-127
            b0 = i * 64
            nc.tensor.matmul(
                sps[:, i, :],
                lhsT=kT[b0:b0 + 64, hc, kc * 128:(kc + 1) * 128],
                rhs=qT[b0:b0 + 64, hc, qsl],
                start=True, stop=True)
        nc.scalar.activation(
            out=P2big[:, g % NP2, :, :], in_=sps[:], func=EXP, scale=0.125)

    for sc in range(NSC):
        emit_s_exp(2 * sc)
        emit_s_exp(2 * sc + 1)
        for j in (2 * sc - AVD, 2 * sc + 1 - AVD):
            if j >= 0:
                emit_av(j)
                if j % 16 == 15:
                    emit_norm(j // 16)
        for fn, args in sched[sc]:
            fn(*args)
    for j in range(NG - AVD, NG):
        emit_av(j)
    emit_norm(15)
    # tail: output projection for the last quarter
    opl = make_op_late(3)
    for s in range(8):
        opl(s)


def build():
    nc = bacc.Bacc("TRN2", target_bir_lowering=False, debug=False, num_devices=8)
    xt_d = nc.dram_tensor("xt", [D, T], BF16, kind="ExternalInput").ap()
    wq_d = nc.dram_tensor("wq", [D, 512], BF16, kind="ExternalInput").ap()
    wk_d = nc.dram_tensor("wk", [D, 512], BF16, kind="ExternalInput").ap()
    wv_d = nc.dram_tensor("wv", [D, 512], BF16, kind="ExternalInput").ap()
    wo_d = nc.dram_tensor("wout", [512, D], BF16, kind="ExternalInput").ap()
    out_d = nc.dram_tensor("out", [T, D], FP32, kind="ExternalOutput").ap()
    with tile.TileContext(nc) as tc:
        with ExitStack() as ctx:
            _body(ctx, nc, tc, xt_d, wq_d, wk_d, wv_d, wo_d, out_d)
    nc.compile()
    return nc


_nc = None


def _get_nc():
    global _nc
    if _nc is None:
        _nc = build()
    return _nc


def make_in_maps(x, Wqkv, Wout):
    bf = ml_dtypes.bfloat16
    in_maps = []
    for c in range(8):
        b, g = divmod(c, 2)
        cs = slice(g * 512, (g + 1) * 512)
        in_maps.append({
            "xt": np.ascontiguousarray(x[b].T).astype(bf),
            "wq": np.ascontiguousarray(Wqkv[:, 0 * D:1 * D][:, cs]).astype(bf),
            "wk": np.ascontiguousarray(Wqkv[:, 1 * D:2 * D][:, cs]).astype(bf),
            "wv": np.ascontiguousarray(Wqkv[:, 2 * D:3 * D][:, cs]).astype(bf),
            "wout": np.ascontiguousarray(Wout[cs, :]).astype(bf),
        })
    return in_maps


def kernel(x, Wqkv, Wout, _trace=False):
    nc = _get_nc()
    x = np.asarray(x, dtype=np.float32)
    Wqkv = np.asarray(Wqkv, dtype=np.float32)
    Wout = np.asarray(Wout, dtype=np.float32)
    in_maps = make_in_maps(x, Wqkv, Wout)
    kwargs = {}
    if _trace:
        kwargs["trace"] = True
    res = run_bass_kernel_spmd(nc, in_maps, core_ids=list(range(8)), **kwargs)
    outs = [res.results[c]["out"] for c in range(8)]
    out = np.stack([outs[2 * b] + outs[2 * b + 1] for b in range(4)])
    if _trace:
        kernel.last_result = res
    return out


# revision 18
# speedup vs baseline: 1.0520x; 1.0104x over previous
"""Multi-head attention (B=4, T=2048, D=1024, H=16) on 8 TRN2 NeuronCores.

Sharding: core c -> (batch b = c//2, head-group g = c%2 of 8 heads).
Each core computes the qkv projection for its batch restricted to its 8
heads, full attention for those heads, and a partial output projection
(ctx_local @ Wout[rows of its heads]).  Host sums the two partials per batch.

Per-core kernel: the PE stream is the bottleneck (~320us incl. the ~105ns
drain paid when entering/leaving the split-tile S-pairs), so the schedule
minimizes matmul class switches: superchunks of 2 attention chunks emit
[S,S][AV x4][one free-512 projection burst].  The two S matmuls of a chunk
run concurrently on disjoint 64-row PE tiles (one per head); exp runs on the
ACT engine (286us total, slack vs PE); AV is deferred 4 chunks through a
circular bf16 P buffer.  The output projection is split by contraction: the
head-pair-0/1 partial runs early (staged bf16), pair-2/3 + finalize after
the last norms, shrinking the tail.  x stays resident in SBUF; DMAs are
issued in first-use order so the first matmul lands ~13us in.
"""

import numpy as np
import ml_dtypes
from contextlib import ExitStack

import concourse.bass as bass
import concourse.bacc as bacc
import concourse.tile as tile
from concourse import mybir
from concourse.bass_utils import run_bass_kernel_spmd

FP32 = mybir.dt.float32
BF16 = mybir.dt.bfloat16
EXP = mybir.ActivationFunctionType.Exp

D = 1024
T = 2048
HPC = 8          # heads per core
FC = 8           # feature chunks of 128 (projection contraction)
KC = 16          # k chunks of 128 per quarter
NG = 256         # total chunks: 4 pairs x 4 quarters x 16
AVD = 4          # AV defer (chunks)
NP2 = 12         # circular exp-output slots


def _norm(nc, rpool, ctx_sb, ctxp, hh, hc, qsl):
    """ctx_sb[hb:hb+64, hc, qsl] = ctxp[0:64] / ctxp[64] (sumexp row)."""
    hb = (hh % 2) * 64
    rtmp = rpool.tile([1, 512], FP32, tag="rtmp")
    nc.vector.tensor_copy(out=rtmp[:], in_=ctxp[64:65, :])
    rt = rpool.tile([1, 512], FP32, tag="rt")
    nc.vector.reciprocal_approx_fast(out=rt[:], in_=rtmp[:])
    rb = rpool.tile([64, 512], FP32, tag="rb")
    nc.gpsimd.partition_broadcast(rb[:], rt[0:1, :], channels=64)
    nc.vector.tensor_mul(ctx_sb[hb:hb + 64, hc, qsl], ctxp[0:64, :], rb[:])


def _body(ctx, nc, tc, xt_d, wq_d, wk_d, wv_d, wo_d, out_d):
    xt_r = xt_d.rearrange("(f p) t -> p f t", p=128)
    persist = ctx.enter_context(tc.tile_pool(name="persist", bufs=1))
    xbig = persist.tile([128, FC, T], BF16, tag="x")
    qT = persist.tile([128, 4, T], BF16, tag="qT")
    kT = persist.tile([128, 4, T], BF16, tag="kT")
    v_sb = persist.tile([128, KC, HPC, 65], BF16, tag="v")
    ctx_sb = persist.tile([128, 4, T], BF16, tag="ctx")
    wq_sb = persist.tile([128, FC, 512], BF16, tag="wq")
    wk_sb = persist.tile([128, FC, 512], BF16, tag="wk")
    wv_sb = persist.tile([128, FC, 512], BF16, tag="wv")
    wo_sb = persist.tile([128, 4, D], BF16, tag="wo")
    P2big = persist.tile([128, NP2, 2, 512], BF16, tag="P2big")
    opstage = persist.tile([128, 16, D], BF16, tag="opstage")
    warm = persist.tile([1, 4], FP32, tag="warm")

    # Preload the ACT exp table-set during the initial DMA wait.
    nc.vector.memset(warm[:], 0.0)
    nc.scalar.activation(out=warm[:], in_=warm[:], func=EXP)

    nc.vector.memset(v_sb[:, :, :, 64:65], 1.0)

    # DMA in first-use order.
    wq_r = wq_d.rearrange("(f p) c -> p f c", p=128)
    wk_r = wk_d.rearrange("(f p) c -> p f c", p=128)
    wv_r = wv_d.rearrange("(f p) c -> p f c", p=128)
    nc.sync.dma_start(out=wq_sb[:, 0:4, :], in_=wq_r[:, 0:4, :])
    nc.sync.dma_start(out=xbig[:, 0:4, 0:512], in_=xt_r[:, 0:4, 0:512])
    nc.sync.dma_start(out=wk_sb[:, 0:4, :], in_=wk_r[:, 0:4, :])
    nc.sync.dma_start(out=wq_sb[:, 4:8, :], in_=wq_r[:, 4:8, :])
    nc.sync.dma_start(out=xbig[:, 4:8, 0:512], in_=xt_r[:, 4:8, 0:512])
    nc.sync.dma_start(out=wk_sb[:, 4:8, :], in_=wk_r[:, 4:8, :])
    nc.sync.dma_start(out=wv_sb[:], in_=wv_r[:])
    nc.sync.dma_start(out=xbig[:, :, 512:1024], in_=xt_r[:, :, 512:1024])
    nc.sync.dma_start(out=xbig[:, :, 1024:1536], in_=xt_r[:, :, 1024:1536])
    nc.sync.dma_start(out=xbig[:, :, 1536:2048], in_=xt_r[:, :, 1536:2048])
    nc.sync.dma_start(out=wo_sb[:], in_=wo_d.rearrange("(c p) d -> p c d", p=128))

    spsum = ctx.enter_context(tc.tile_pool(name="spsum", bufs=2, space="PSUM"))
    cpsum = ctx.enter_context(tc.tile_pool(name="cpsum", bufs=2, space="PSUM"))
    # 8 PSUM banks: spsum 4, ctx ring 2, unified projection ring 2.
    ps_u = ctx.enter_context(tc.tile_pool(name="ps_u", bufs=2, space="PSUM"))
    rpool = ctx.enter_context(tc.tile_pool(name="rpool", bufs=2))
    osb = ctx.enter_context(tc.tile_pool(name="osb", bufs=2))

    # ---- projection bursts (each one free-512 burst + one copy) ----

    def qk_burst(pair, ts, qk):
        """Full-span projection of one (pair, span, q-or-k)."""
        tsl = slice(ts * 512, (ts + 1) * 512)
        w_sb, dst = ((wq_sb, qT), (wk_sb, kT))[qk]
        p = ps_u.tile([128, 512], FP32, tag="acc", name="qkp")
        for fc in range(FC):
            nc.tensor.matmul(
                p[:],
                lhsT=w_sb[:, fc, pair * 128:(pair + 1) * 128],
                rhs=xbig[:, fc, tsl],
                start=(fc == 0), stop=(fc == FC - 1))
        nc.vector.tensor_copy(out=dst[:, pair, tsl], in_=p[:])

    def v_burst(kc, lo):
        """v projection for k-chunk kc, heads 0-3 (lo) or 4-7."""
        csl = slice(0, 256) if lo else slice(256, 512)
        psv = ps_u.tile([128, 512], FP32, tag="acc", name="psv")
        xc = slice((kc // 4) * 512 + (kc % 4) * 128,
                   (kc // 4) * 512 + (kc % 4) * 128 + 128)
        for fc in range(FC):
            nc.tensor.matmul(
                psv[:, 0:256],
                lhsT=xbig[:, fc, xc],
                rhs=wv_sb[:, fc, csl],
                start=(fc == 0), stop=(fc == FC - 1))
        h0 = 0 if lo else 4
        nc.vector.tensor_copy(
            out=v_sb[:, kc, h0:h0 + 4, 0:64],
            in_=psv[:, 0:256].rearrange("p (h d) -> p h d", h=4))

    def make_op_early(q):
        """cc 0-1 partial of (tcg, col-half), staged to SBUF bf16; runnable
        once head-pairs 0 and 1 have normed quarter q."""
        def unit(s):
            unit_i, ch = divmod(s, 2)
            tcg = q * 4 + unit_i
            csl = slice(ch * 512, (ch + 1) * 512)
            po = ps_u.tile([128, 512], FP32, tag="acc", name="poe")
            for cc in (0, 1):
                nc.tensor.matmul(
                    po[:],
                    lhsT=ctx_sb[:, cc, tcg * 128:(tcg + 1) * 128],
                    rhs=wo_sb[:, cc, csl],
                    start=(cc == 0), stop=(cc == 1))
            nc.vector.tensor_copy(out=opstage[:, tcg, csl], in_=po[:])
        return unit

    def make_op_mid(q):
        """cc 2 partial accumulated into the bf16 stage; needs pair 2."""
        def unit(s):
            unit_i, ch = divmod(s, 2)
            tcg = q * 4 + unit_i
            csl = slice(ch * 512, (ch + 1) * 512)
            po = ps_u.tile([128, 512], FP32, tag="acc", name="pom")
            nc.tensor.matmul(
                po[:],
                lhsT=ctx_sb[:, 2, tcg * 128:(tcg + 1) * 128],
                rhs=wo_sb[:, 2, csl],
                start=True, stop=True)
            nc.vector.tensor_add(
                out=opstage[:, tcg, csl], in0=opstage[:, tcg, csl], in1=po[:])
        return unit

    def make_op_late(q):
        """cc 3 partial + staged add + DMA; needs head-pair 3's norm."""
        st = {"ot": None}

        def unit(s):
            unit_i, ch = divmod(s, 2)
            tcg = q * 4 + unit_i
            csl = slice(ch * 512, (ch + 1) * 512)
            if ch == 0:
                st["ot"] = osb.tile([128, D], FP32, tag="ot", name="ot")
            po = ps_u.tile([128, 512], FP32, tag="acc", name="pol")
            nc.tensor.matmul(
                po[:],
                lhsT=ctx_sb[:, 3, tcg * 128:(tcg + 1) * 128],
                rhs=wo_sb[:, 3, csl],
                start=True, stop=True)
            nc.vector.tensor_add(
                out=st["ot"][:, csl], in0=opstage[:, tcg, csl], in1=po[:])
            if ch == 1:
                nc.sync.dma_start(
                    out=out_d[tcg * 128:(tcg + 1) * 128, :],
                    in_=st["ot"][:])
        return unit

    # ---- schedule: one burst list per superchunk of 2 chunks ----

    NSC = NG // 2
    sched = {s: [] for s in range(NSC)}
    extra = [0.0] * NSC

    def place(start, deadline, cost, fn, args, after=None):
        lo = max(start, after if after is not None else 0, 0)
        hi = min(deadline, NSC - 1)
        s = min(range(lo, hi + 1), key=lambda x: (extra[x], -x))
        sched[s].append((fn, args))
        extra[s] += cost
        return s

    # v heads 0-3: JIT in quarter 0 (AV defer 4); heads 4-7 by chunk 128.
    for kc in range(KC):
        sched[kc // 2].append((v_burst, (kc, True)))
        extra[kc // 2] += 1000
    for kc in range(KC):
        place(32, 62, 1000, v_burst, (kc, False))
    # pair 0 remaining spans: k-span ts by chunk 4ts, q-span ts by 16ts.
    for ts in range(1, 4):
        place(0, 2 * ts - 1, 1930, qk_burst, (0, ts, 1))
    for ts in range(1, 4):
        place(1, 8 * ts - 2, 1930, qk_burst, (0, ts, 0))
    # pairs 1-3: k-span ts by chunk 64p+4ts, q-span ts by 64p+16ts.
    for pair in range(1, 4):
        for ts in range(4):
            place(3, 32 * pair + 2 * ts - 2, 1930, qk_burst, (pair, ts, 1))
        for ts in range(4):
            place(3, 32 * pair + 8 * ts - 2, 1930, qk_burst, (pair, ts, 0))
    # output projection cc0-1 after head-pair 1's norm (chunk ~82+16q);
    # cc2-3 + finalize after head-pair 3's norm (chunk ~210+16q).
    for q in range(4):
        ope = make_op_early(q)
        opm = make_op_mid(q)
        for s in range(8):
            ge = place(43 + 8 * q, 101, 650, ope, (s,))
            place(max(75 + 8 * q, ge), 104, 450, opm, (s,), after=ge)
    for q in range(3):
        opl = make_op_late(q)
        sp = None
        for s in range(8):
            sp = place(107 + 8 * q, min(106 + 8 * (q + 1), NSC - 1), 450,
                       opl, (s,), after=sp)

    # ---- the flat attention pipeline ----

    def qk_burst_fchalf(qk, fchalf):
        tsl = slice(0, 512)
        w_sb, dst = ((wq_sb, qT), (wk_sb, kT))[qk]
        p = startacc[qk]
        for fc in range(4 * fchalf, 4 * fchalf + 4):
            nc.tensor.matmul(
                p[:],
                lhsT=w_sb[:, fc, 0:128],
                rhs=xbig[:, fc, tsl],
                start=(fc == 0), stop=(fc == FC - 1))
        if fchalf == 1:
            nc.vector.tensor_copy(out=dst[:, 0, tsl], in_=p[:])

    startacc = [ps_u.tile([128, 512], FP32, tag="acc", name="sa0"),
                ps_u.tile([128, 512], FP32, tag="acc", name="sa1")]
    qk_burst_fchalf(0, 0)
    qk_burst_fchalf(1, 0)
    qk_burst_fchalf(0, 1)
    qk_burst_fchalf(1, 1)

    qinfo = {}

    def emit_av(j):
        info = qinfo[j // 16]
        kc = j % 16
        for i, ctxp in ((0, info["ctxA"]), (1, info["ctxB"])):
            nc.tensor.matmul(
                ctxp[:],
                lhsT=v_sb[:, kc, 2 * info["hc"] + i, :],
                rhs=P2big[:, j % NP2, i, :],
                start=(kc == 0), stop=(kc == KC - 1))

    def emit_norm(q):
        info = qinfo[q]
        qsl = slice(info["qq"] * 512, (info["qq"] + 1) * 512)
        _norm(nc, rpool, ctx_sb, info["ctxA"], 2 * info["hc"], info["hc"], qsl)
        _norm(nc, rpool, ctx_sb, info["ctxB"], 2 * info["hc"] + 1,
              info["hc"], qsl)
        del qinfo[q]

    def emit_s_exp(g):
        hc, qq, kc = g // 64, (g // 16) % 4, g % 16
        if kc == 0:
            qinfo[g // 16] = {
                "hc": hc, "qq": qq,
                "ctxA": cpsum.tile([65, 512], FP32, tag="ctx", name="ctxA"),
                "ctxB": cpsum.tile([65, 512], FP32, tag="ctx", name="ctxB"),
            }
        qsl = slice(qq * 512, (qq + 1) * 512)
        sps = spsum.tile([128, 2, 512], FP32, tag="S")
        for i in range(2):          # head A on rows 0-63, head B on 64-127
            b0 = i * 64
            nc.tensor.matmul(
                sps[:, i, :],
                lhsT=kT[b0:b0 + 64, hc, kc * 128:(kc + 1) * 128],
                rhs=qT[b0:b0 + 64, hc, qsl],
                start=True, stop=True)
        nc.scalar.activation(
            out=P2big[:, g % NP2, :, :], in_=sps[:], func=EXP, scale=0.125)

    for sc in range(NSC):
        emit_s_exp(2 * sc)
        emit_s_exp(2 * sc + 1)
        for j in (2 * sc - AVD, 2 * sc + 1 - AVD):
            if j >= 0:
                emit_av(j)
                if j % 16 == 15:
                    emit_norm(j // 16)
        for fn, args in sched[sc]:
            fn(*args)
    for j in range(NG - AVD, NG):
        emit_av(j)
    # tail: norm the last quarter in token halves, each immediately
    # followed by its output-projection units.
    info = qinfo[15]
    opl = make_op_late(3)
    for half in range(2):
        tsl = slice(1536 + half * 256, 1536 + (half + 1) * 256)
        psl = slice(half * 256, (half + 1) * 256)
        for i, ctxp in ((0, info["ctxA"]), (1, info["ctxB"])):
            hb = i * 64
            rtmp = rpool.tile([1, 512], FP32, tag="rtmp", name="rtmp")
            nc.vector.tensor_copy(out=rtmp[:, 0:256], in_=ctxp[64:65, psl])
            rt = rpool.tile([1, 512], FP32, tag="rt", name="rt")
            nc.vector.reciprocal_approx_fast(out=rt[:, 0:256], in_=rtmp[:, 0:256])
            rb = rpool.tile([64, 512], FP32, tag="rb", name="rb")
            nc.gpsimd.partition_broadcast(rb[:, 0:256], rt[0:1, 0:256], channels=64)
            nc.vector.tensor_mul(
                ctx_sb[hb:hb + 64, 3, tsl], ctxp[0:64, psl], rb[:, 0:256])
        for s in (4 * half, 4 * half + 1, 4 * half + 2, 4 * half + 3):
            opl(s)
    del qinfo[15]


def build():
    nc = bacc.Bacc("TRN2", target_bir_lowering=False, debug=False, num_devices=8)
    xt_d = nc.dram_tensor("xt", [D, T], BF16, kind="ExternalInput").ap()
    wq_d = nc.dram_tensor("wq", [D, 512], BF16, kind="ExternalInput").ap()
    wk_d = nc.dram_tensor("wk", [D, 512], BF16, kind="ExternalInput").ap()
    wv_d = nc.dram_tensor("wv", [D, 512], BF16, kind="ExternalInput").ap()
    wo_d = nc.dram_tensor("wout", [512, D], BF16, kind="ExternalInput").ap()
    out_d = nc.dram_tensor("out", [T, D], FP32, kind="ExternalOutput").ap()
    with tile.TileContext(nc) as tc:
        with ExitStack() as ctx:
            _body(ctx, nc, tc, xt_d, wq_d, wk_d, wv_d, wo_d, out_d)
    nc.compile()
    return nc


_nc = None


def _get_nc():
    global _nc
    if _nc is None:
        _nc = build()
    return _nc


def make_in_maps(x, Wqkv, Wout):
    bf = ml_dtypes.bfloat16
    in_maps = []
    for c in range(8):
        b, g = divmod(c, 2)
        cs = slice(g * 512, (g + 1) * 512)
        in_maps.append({
            "xt": np.ascontiguousarray(x[b].T).astype(bf),
            "wq": np.ascontiguousarray(Wqkv[:, 0 * D:1 * D][:, cs]).astype(bf),
            "wk": np.ascontiguousarray(Wqkv[:, 1 * D:2 * D][:, cs]).astype(bf),
            "wv": np.ascontiguousarray(Wqkv[:, 2 * D:3 * D][:, cs]).astype(bf),
            "wout": np.ascontiguousarray(Wout[cs, :]).astype(bf),
        })
    return in_maps


def kernel(x, Wqkv, Wout, _trace=False):
    nc = _get_nc()
    x = np.asarray(x, dtype=np.float32)
    Wqkv = np.asarray(Wqkv, dtype=np.float32)
    Wout = np.asarray(Wout, dtype=np.float32)
    in_maps = make_in_maps(x, Wqkv, Wout)
    kwargs = {}
    if _trace:
        kwargs["trace"] = True
    res = run_bass_kernel_spmd(nc, in_maps, core_ids=list(range(8)), **kwargs)
    outs = [res.results[c]["out"] for c in range(8)]
    out = np.stack([outs[2 * b] + outs[2 * b + 1] for b in range(4)])
    if _trace:
        kernel.last_result = res
    return out
